# revision 23
# baseline (speedup 1.0000x reference)
"""SSIM loss kernel for Trainium2 (8 NeuronCores, data-parallel over batch).

Math (per image pair, window=3x3 uniform stride 3, pad 1):
  box sums S1=sum(x), S2=sum(y), P=sum(x^2), Q=sum(y^2), R=sum(xy) over each
  disjoint 3x3 window (top/left zero pad).  With w = S1*S2:
    ssim = (2w + 81*C1)(18R - 2w + 81*C2)
         / ((S1^2 + S2^2 + 81*C1)(9(P+Q) - S1^2 - S2^2 + 81*C2))
  output = mean over all windows and batch.

Box reduction runs on the TensorEngine: lhsT is a 0/1 group-indicator
matrix (H groups of 3 rows -> psum partitions), rhs is the image (or
product) tile with a stride-3 column AP; three column-shifted matmuls
accumulate in PSUM so the full 3x3 box sum appears with zero vector work.
"""

import numpy as np

import concourse.bass as bass
import concourse.tile as tile
from concourse import mybir
from concourse.bass_utils import run_bass_kernel_spmd

F32 = mybir.dt.float32
BF16 = mybir.dt.float16  # fp16: 10 mantissa bits, exact for 0/1 weights

H = 2048
W = 2048
G = 683            # output groups per dim
B = 8
NCORES = 8
C1 = 0.01 ** 2
C2 = 0.03 ** 2
B81C1 = 81.0 * C1  # 0.0081
B81C2 = 81.0 * C2  # 0.0729

# H blocks: (row_start, nrows, a_name).  Block 0 drops the zero pad row.
BLOCKS = [(0, 125, "a_first")]
for t in range(1, 16):
    BLOCKS.append((126 * t - 1, 126, None))  # a variant chosen by span position
BLOCKS.append((2015, 33, "a_tail"))

SPANS = [[t] for t in range(17)]
PSUM_BASE = [0]           # psum base partition by position-in-span
# valid (group-row) slices within the 128 psum partitions per span kind
VALID_FULL = [(0, 42)]
VALID_TAIL = [(0, 11)]


def _make_a_mats():
    import ml_dtypes
    mats = {}
    a = np.zeros((125, 64), np.float32)
    for k in range(125):
        a[k, (k + 1) // 3] = 1.0
    mats["a_first"] = a
    a = np.zeros((126, 64), np.float32)
    for k in range(126):
        a[k, k // 3] = 1.0
    mats["a_mid"] = a
    a = np.zeros((33, 64), np.float32)
    for k in range(33):
        a[k, k // 3] = 1.0
    mats["a_tail"] = a
    return {k: v.astype(np.float16) for k, v in mats.items()}


A_MATS = _make_a_mats()

# (chunk psum width, rhs j-slices per shift). chunk1 covers out cols j 0:512,
# chunk2 covers j 427:683 (first 85 cols overlap chunk1 and are ignored).
# Each entry: list of (k_index_into_3, j_lo, j_hi, out_lo, out_hi)
CHUNKS = [
    # (psum_cols, used_lo, used_hi, shifts)
    (512, 0, 512, [(0, 0, 512, 0, 512),      # col 3j
                   (1, 0, 512, 0, 512),      # col 3j+1
                   (2, 0, 511, 1, 512)]),    # col 3j-1 = 3(j-1)+2, j>=1
    (171, 0, 171, [(0, 512, 683, 0, 171),
                   (1, 512, 683, 0, 171),
                   (2, 511, 682, 0, 171)]),
]


def _build_nc():
    nc = bass.Bass()
    img1_d = nc.dram_tensor("img1", [H, W], F32, kind="ExternalInput")
    img2_d = nc.dram_tensor("img2", [H, W], F32, kind="ExternalInput")
    a_d = {}
    for name, arr in A_MATS.items():
        a_d[name] = nc.dram_tensor(name, list(arr.shape), BF16,
                                   kind="ExternalInput")
    out_d = nc.dram_tensor("out", [128, 1], F32, kind="ExternalOutput")

    with tile.TileContext(nc) as tc:
        with (
            tc.tile_pool(name="singles", bufs=1) as singles,
            tc.tile_pool(name="imgs", bufs=4) as imgs,
            tc.tile_pool(name="prods", bufs=5) as prods,
            tc.tile_pool(name="maps", bufs=3) as maps,
            tc.tile_pool(name="psum", bufs=4, space="PSUM") as psum,
        ):
            # constants
            a_t = {}
            for name, arr in A_MATS.items():
                t = singles.tile(list(arr.shape), BF16, tag=name)
                nc.sync.dma_start(out=t, in_=a_d[name][:, :])
                a_t[name] = t
            acc = singles.tile([128, 1], F32, tag="acc")
            nc.vector.memset(acc, 0.0)
            zero_c = singles.tile([128, 1], F32, tag="zero_c")
            nc.vector.memset(zero_c, 0.0)
            c1_c = singles.tile([128, 1], F32, tag="c1_c")
            nc.vector.memset(c1_c, B81C1)
            c2_c = singles.tile([128, 1], F32, tag="c2_c")
            nc.vector.memset(c2_c, B81C2)

            for si, span in enumerate(SPANS):
                # ---- load inputs + full-res products for this span ----
                blk = []
                for pos, t_idx in enumerate(span):
                    r0, nr, a_name = BLOCKS[t_idx]
                    if a_name is None:
                        a_name = "a_mid"
                    x_t = imgs.tile([126, 2049], BF16, tag="x")
                    y_t = imgs.tile([126, 2049], BF16, tag="y")
                    nc.gpsimd.dma_start(out=x_t[:nr, 0:W], in_=img1_d[r0:r0 + nr, :])
                    nc.gpsimd.dma_start(out=y_t[:nr, 0:W], in_=img2_d[r0:r0 + nr, :])
                    xy_t = prods.tile([126, 2049], BF16, tag="xy")
                    xs_t = prods.tile([126, 2049], BF16, tag="xs")
                    ys_t = prods.tile([126, 2049], BF16, tag="ys")
                    nc.vector.tensor_mul(xy_t[:nr, 0:W], x_t[:nr, 0:W], y_t[:nr, 0:W])
                    nc.scalar.activation(
                        out=xs_t[:nr, 0:W], in_=x_t[:nr, 0:W],
                        func=mybir.ActivationFunctionType.Square,
                        bias=zero_c[:nr, :], scale=1.0)
                    # y^2 on DVE (fp16 self-mul, 2x mode) — cost-model
                    # trace shows ScalarE as the bottleneck engine (194us
                    # busy / 203us span), so rebalance one square to DVE.
                    nc.vector.tensor_mul(ys_t[:nr, 0:W], y_t[:nr, 0:W],
                                         y_t[:nr, 0:W])
                    blk.append((pos, nr, a_name, x_t, y_t, xy_t, xs_t, ys_t))

                full_span = span[0] < 16
                n_parts = 64  # psum partitions written
                valid = VALID_FULL if full_span else VALID_TAIL

                def mm_quantity(src_idx, tag):
                    """Emit the 3-shift box matmuls for one quantity.
                    src_idx selects tile (3=x,4=y,5=xy,6=xs,7=ys)."""
                    c1 = psum.tile([128, 512], F32, tag="pc1")
                    c2 = psum.tile([128, 171], F32, tag="pc2")
                    for ci, (pw, _ulo, _uhi, shifts) in enumerate(CHUNKS):
                        dst = c1 if ci == 0 else c2
                        first = True
                        for pos, nr, a_name, *tiles in blk:
                            a_ap = a_t[a_name]
                            m = a_ap.shape[1]
                            base = PSUM_BASE[pos]
                            src = tiles[src_idx - 3]
                            r3 = src.rearrange(
                                "p (j three) -> p j three", three=3)
                            nlast = len(shifts) - 1
                            for shi, (kk, jlo, jhi, olo, ohi) in enumerate(shifts):
                                nc.tensor.matmul(
                                    out=dst[base:base + m, olo:ohi],
                                    lhsT=a_ap,
                                    rhs=r3[:nr, jlo:jhi, kk],
                                    start=(first and pos == 0),
                                    stop=(shi == nlast and pos == len(blk) - 1),
                                )
                                first = False
                    return c1, c2

                ps1 = mm_quantity(3, "s1")
                ps2 = mm_quantity(4, "s2")

                # ---- map stage part 1: consume S1/S2 asap to free psum ----
                pm = n_parts
                chunk_views = []
                for ci, (pw, ulo, uhi, _s) in enumerate(CHUNKS):
                    fd = uhi - ulo
                    s1c = ps1[ci][0:pm, ulo:uhi]
                    s2c = ps2[ci][0:pm, ulo:uhi]
                    s2s = maps.tile([128, 512], F32, tag="s2s")
                    u_t = maps.tile([128, 512], F32, tag="u")
                    v_t = maps.tile([128, 512], F32, tag="v")
                    w_t = maps.tile([128, 512], F32, tag="w")
                    nc.scalar.copy(out=s2s[:pm, :fd], in_=s2c)
                    nc.scalar.activation(
                        out=u_t[:pm, :fd], in_=s1c,
                        func=mybir.ActivationFunctionType.Square,
                        bias=zero_c[:pm, :], scale=1.0)
                    nc.scalar.activation(
                        out=v_t[:pm, :fd], in_=s2c,
                        func=mybir.ActivationFunctionType.Square,
                        bias=zero_c[:pm, :], scale=1.0)
                    nc.vector.tensor_mul(w_t[:pm, :fd], s1c, s2s[:pm, :fd])
                    chunk_views.append((fd, u_t, v_t, w_t))

                pp = mm_quantity(6, "p")
                qq = mm_quantity(7, "q")
                rr = mm_quantity(5, "r")

                # ---- map stage part 2 ----
                for ci, (pw, ulo, uhi, _s) in enumerate(CHUNKS):
                    fd, u_t, v_t, w_t = chunk_views[ci]
                    p_c = pp[ci][0:pm, ulo:uhi]
                    q_c = qq[ci][0:pm, ulo:uhi]
                    r_c = rr[ci][0:pm, ulo:uhi]
                    qs = maps.tile([128, 512], F32, tag="qs")
                    pq = maps.tile([128, 512], F32, tag="pq")
                    n1 = maps.tile([128, 512], F32, tag="n1")
                    n2 = maps.tile([128, 512], F32, tag="n2")
                    d1 = maps.tile([128, 512], F32, tag="d1")
                    d2 = maps.tile([128, 512], F32, tag="d2")
                    num = maps.tile([128, 512], F32, tag="num")
                    den = maps.tile([128, 512], F32, tag="den")
                    rcp = maps.tile([128, 512], F32, tag="rcp")
                    scr = maps.tile([128, 512], F32, tag="scr")
                    part = maps.tile([128, 1], F32, tag="part")

                    nc.scalar.copy(out=qs[:pm, :fd], in_=q_c)
                    nc.vector.tensor_add(pq[:pm, :fd], p_c, qs[:pm, :fd])
                    addop = mybir.AluOpType.add
                    idf = mybir.ActivationFunctionType.Identity
                    # N1 = 2w + 81C1   (ScalarE: affine via Identity)
                    nc.scalar.activation(out=n1[:pm, :fd], in_=w_t[:pm, :fd],
                                         func=idf, bias=c1_c[:pm, :], scale=2.0)
                    # N2 = (18R + 81C2) - 2w
                    n2a = maps.tile([128, 512], F32, tag="n2a")
                    w2t = maps.tile([128, 512], F32, tag="w2t")
                    nc.scalar.activation(out=n2a[:pm, :fd], in_=r_c,
                                         func=idf, bias=c2_c[:pm, :], scale=18.0)
                    nc.vector.tensor_scalar_mul(w2t[:pm, :fd], w_t[:pm, :fd], 2.0)
                    nc.vector.tensor_sub(n2[:pm, :fd], n2a[:pm, :fd], w2t[:pm, :fd])
                    # D1 = (u + v) + 81C1 ; D2 = (9pq + 81C2) - (u + v)
                    upv = maps.tile([128, 512], F32, tag="upv")
                    pq9 = maps.tile([128, 512], F32, tag="pq9")
                    nc.vector.tensor_add(upv[:pm, :fd], u_t[:pm, :fd], v_t[:pm, :fd])
                    nc.scalar.activation(out=d1[:pm, :fd], in_=upv[:pm, :fd],
                                         func=idf, bias=c1_c[:pm, :], scale=1.0)
                    nc.scalar.activation(out=pq9[:pm, :fd], in_=pq[:pm, :fd],
                                         func=idf, bias=c2_c[:pm, :], scale=9.0)
                    nc.vector.tensor_sub(d2[:pm, :fd], pq9[:pm, :fd], upv[:pm, :fd])
                    nc.vector.tensor_mul(num[:pm, :fd], n1[:pm, :fd], n2[:pm, :fd])
                    nc.vector.tensor_mul(den[:pm, :fd], d1[:pm, :fd], d2[:pm, :fd])
                    # ScalarE LUT reciprocal (~1 elem/cycle/lane vs DVE's
                    # iterative ~8 cyc/elem); accuracy ~1e-3 is fine at our
                    # 2e-2 tolerance. bass's wrapper refuses Reciprocal, so
                    # emit the InstActivation directly (bias/scale/alpha as
                    # immediates, the Copy/Reciprocal form).
                    nc.scalar.add_instruction(mybir.InstActivation(
                        name=nc.get_next_instruction_name(),
                        func=mybir.ActivationFunctionType.Reciprocal,
                        ins=[nc.scalar.lower_ap(den[:pm, :fd]),
                             mybir.ImmediateValue(dtype=F32, value=0.0),
                             mybir.ImmediateValue(dtype=F32, value=1.0),
                             mybir.ImmediateValue(dtype=F32, value=0.0)],
                        outs=[nc.scalar.lower_ap(rcp[:pm, :fd])]))
                    nc.vector.tensor_mul(scr[:pm, :fd], rcp[:pm, :fd],
                                         num[:pm, :fd])
                    nc.vector.tensor_reduce(out=part[:pm, :], in_=scr[:pm, :fd],
                                            axis=mybir.AxisListType.X,
                                            op=addop)
                    for vlo, vhi in valid:
                        nc.vector.tensor_add(acc[vlo:vhi, :], acc[vlo:vhi, :],
                                             part[vlo:vhi, :])

            nc.sync.dma_start(out=out_d[:, :], in_=acc)
    _split_excess_waits(nc)
    return nc


def _split_excess_waits(nc):
    """Walrus codegen caps compute/DMA instructions at ONE sync wait
    (EventSemaphore carriers hold two).  Move excess waits onto injected
    same-engine InstEventSemaphore instructions immediately preceding the
    over-budget instruction; the engine executes its stream in order, so
    blocking semantics are identical."""
    for f in nc.m.functions:
        for bb in f.blocks:
            changed = False
            new_insts = []
            for inst in bb.instructions:
                si = inst.sync_info
                if (si is not None and si.on_wait and len(si.on_wait) > 1
                        and not isinstance(inst, mybir.InstEventSemaphore)):
                    waits = list(si.on_wait)
                    extra, keep = waits[:-1], waits[-1:]
                    for i, w in enumerate(extra):
                        ev = mybir.InstNoOp(
                            name="I-evw-%s-%d" % (inst.name, i),
                            sync_info=mybir.SyncInfo(on_wait=[w], on_update=[]),
                            bass_nofuse=True,
                            engine=inst.engine,
                        )
                        new_insts.append(ev)
                    inst.sync_info = mybir.SyncInfo(
                        on_wait=keep, on_update=list(si.on_update))
                    changed = True
                new_insts.append(inst)
            if changed:
                try:
                    bb.instructions = new_insts
                except Exception:
                    del bb.instructions[:]
                    bb.instructions.extend(new_insts)


_NC_CACHE = {}


def _get_nc():
    if "nc" not in _NC_CACHE:
        _NC_CACHE["nc"] = _build_nc()
    return _NC_CACHE["nc"]


def _run(img1, img2, **spmd_kwargs):
    nc = _get_nc()
    img1 = np.ascontiguousarray(np.asarray(img1, np.float32).reshape(B, H, W))
    img2 = np.ascontiguousarray(np.asarray(img2, np.float32).reshape(B, H, W))
    in_maps = []
    for c in range(NCORES):
        m = {"img1": img1[c], "img2": img2[c]}
        for name, arr in A_MATS.items():
            m[name] = arr
        in_maps.append(m)
    res = run_bass_kernel_spmd(nc, in_maps, core_ids=list(range(NCORES)),
                               **spmd_kwargs)
    parts = np.stack([r["out"] for r in res.results])  # [8,128,1]
    total = parts.astype(np.float64).sum()
    val = np.float32(total / (B * G * G))
    return np.asarray(val, np.float32), res


def kernel(img1, img2, window=None, **unused):
    out, _ = _run(img1, img2)
    return out
